# revision 1
# baseline (speedup 1.0000x reference)
"""Multi-head attention (B=2, N=2048, C=1024, H=16, qk-RMSNorm) on 8 TRN2 cores.

Sharding: tensor-parallel over heads x data-parallel over batch.
Core cid handles batch b = cid // 4 and head group g = cid % 4 (4 heads,
c_local = 256 channels). Each core computes qkv for its heads, per-head
RMSNorm on q/k, full softmax attention for its (b, heads), and a partial
output projection against its 256 rows of w_proj. The host sums the 4
partials per batch (TP unshard) and stacks the 2 batches.

All matmuls run as float32r (full-rate fp32, ~1.5e-4 rel err). Attention
works in transposed layout (channels on partitions): scores S^T = k @ q^T,
softmax denominators come free from a ones-column appended to V, and the
1/sum normalization is applied via a PE ones-broadcast + DVE multiply.
exp needs no max-subtraction: q/k rows are RMS-normalized so
|q.k|*scale <= sqrt(64)*sqrt(64)/8 = 8 and exp(8) is safely finite.
"""

import sys

if "/opt/trn_rl_repo" not in sys.path:
    sys.path.insert(0, "/opt/trn_rl_repo")

from contextlib import ExitStack

import numpy as np

import concourse.mybir as mybir
import concourse.tile as tile
from concourse import bacc
from concourse.bass_utils import run_bass_kernel_spmd

F32 = mybir.dt.float32
F32R = mybir.dt.float32r
BF16 = mybir.dt.bfloat16
AF = mybir.ActivationFunctionType

# Problem constants (hardcoded per contract)
B, N, C, H = 2, 2048, 1024, 16
D = C // H          # 64
EPS = 1e-6
NCORES = 8
GROUPS = 4          # head groups (cores per batch)
HL = H // GROUPS    # heads per core = 4
CL = HL * D         # local channels = 256
SCALE = D ** -0.5   # 0.125

# Tiling
P = 128             # partition dim
KT = C // P         # 8 contraction tiles over C
NQ = 512            # query-block (free dim of S^T / AV matmuls)
NB = N // P         # 16 key blocks of 128
NJ = N // NQ        # 8 query blocks per head
HPB = P // D        # heads per 128-channel block = 2
VW = D + 1          # 65: v columns + ones column


def build(n=N, nq=NQ, debug_dump=False, interleave=True, bf16_attn=False):
    """Build the SPMD Bass module. n = sequence length (for scaled tests)."""
    nb = n // P
    nj = n // nq
    kt = KT

    nc = bacc.Bacc("TRN2", target_bir_lowering=False, debug=False,
                   num_devices=NCORES)

    xT_d = nc.dram_tensor("xT", [C, n], F32, kind="ExternalInput").ap()
    wqk_d = nc.dram_tensor("w_qk", [C, 2 * CL], F32, kind="ExternalInput").ap()
    wv_d = nc.dram_tensor("w_v", [C, CL], F32, kind="ExternalInput").ap()
    wpr_d = nc.dram_tensor("w_pr", [CL, C], F32, kind="ExternalInput").ap()
    bqk_d = nc.dram_tensor("b_qk", [P, 4], F32, kind="ExternalInput").ap()
    bv_d = nc.dram_tensor("b_v", [1, CL], F32, kind="ExternalInput").ap()
    bpr_d = nc.dram_tensor("b_pr", [1, C], F32, kind="ExternalInput").ap()
    qkw_d = nc.dram_tensor("qkw", [P, 4], F32, kind="ExternalInput").ap()
    out_d = nc.dram_tensor("out", [n, C], F32, kind="ExternalOutput").ap()
    if debug_dump:
        dqk_d = nc.dram_tensor("dbg_qkT", [4, P, n], F32, kind="ExternalOutput").ap()
        dva_d = nc.dram_tensor("dbg_va", [len(range(n // P)), P, HL * VW], F32, kind="ExternalOutput").ap()
        dat_d = nc.dram_tensor("dbg_at", [CL // P, P, n], F32, kind="ExternalOutput").ap()

    with tile.TileContext(nc) as tc, ExitStack() as ctx:
        con = ctx.enter_context(tc.tile_pool(name="con", bufs=1))
        wp = ctx.enter_context(tc.tile_pool(name="wp", bufs=1))
        qk = ctx.enter_context(tc.tile_pool(name="qk", bufs=1))
        vp = ctx.enter_context(tc.tile_pool(name="vp", bufs=1))
        ps = ctx.enter_context(tc.tile_pool(name="ps", bufs=2, space="PSUM"))
        stg1 = ExitStack()
        xp = stg1.enter_context(tc.tile_pool(name="xp", bufs=1))
        sqp = stg1.enter_context(tc.tile_pool(name="sqp", bufs=1))
        rp = stg1.enter_context(tc.tile_pool(name="rp", bufs=2))

        # ---- constants ----
        ones_f = con.tile([P, P], F32, tag="onesf")
        nc.vector.memset(ones_f[:], 1.0)
        ones_c = con.tile([P, 1], F32R, tag="onesc")   # column of ones (lhsT for sumsq)
        nc.vector.tensor_copy(ones_c[:], ones_f[:, 0:1])
        ones_r = con.tile([1, P], F32R, tag="onesr")   # row of ones (lhsT for bcast/bias)
        nc.vector.tensor_copy(ones_r[:], ones_f[0:1, :])
        ones_m = con.tile([P, P], F32R, tag="onesm")    # ones matrix (lhsT for wide sumsq)
        nc.vector.tensor_copy(ones_m[:], ones_f[:])

        eps_sb = con.tile([P, 1], F32, tag="eps")
        nc.vector.memset(eps_sb[:], EPS)

        bqk_sb = con.tile([P, 4], F32, tag="bqk")
        nc.sync.dma_start(bqk_sb[:], bqk_d[:])
        qkw_sb = con.tile([P, 4], F32, tag="qkw")
        nc.sync.dma_start(qkw_sb[:], qkw_d[:])
        bv_sb = con.tile([1, CL], F32R, tag="bv")
        nc.sync.dma_start(bv_sb[:], bv_d[:].bitcast(F32R))
        bpr_sb = con.tile([1, C], F32R, tag="bpr")
        nc.sync.dma_start(bpr_sb[:], bpr_d[:].bitcast(F32R))

        # ---- weight / input loads ----
        wqk_sb = [wp.tile([P, 2 * CL], F32R, tag=f"wqk{k}", name=f"wqk{k}") for k in range(kt)]
        wv_sb = [wp.tile([P, CL], F32R, tag=f"wv{k}", name=f"wv{k}") for k in range(kt)]
        wpr_sb = [wp.tile([P, C], F32R, tag=f"wpr{k}", name=f"wpr{k}") for k in range(CL // P)]
        xT_sb = [xp.tile([P, n], F32R, tag=f"xt{k}", name=f"xt{k}") for k in range(kt)]
        for k in range(kt):
            nc.sync.dma_start(xT_sb[k][:], xT_d[k * P:(k + 1) * P, :].bitcast(F32R))
            nc.sync.dma_start(wqk_sb[k][:], wqk_d[k * P:(k + 1) * P, :].bitcast(F32R))
            nc.sync.dma_start(wv_sb[k][:], wv_d[k * P:(k + 1) * P, :].bitcast(F32R))
        for k in range(CL // P):
            nc.sync.dma_start(wpr_sb[k][:], wpr_d[k * P:(k + 1) * P, :].bitcast(F32R))

        # ---- stage 1a: qkT = (x @ w_qk)^T in [channel, n] layout ----
        # 4 channel blocks of 128: q(heads01), q(heads23), k(heads01), k(heads23)
        qpool = xp if bf16_attn else qk
        qkT = [qpool.tile([P, n], F32R, tag=f"qkT{m}", name=f"qkT{m}") for m in range(4)]
        if bf16_attn:
            qkB = [qk.tile([P, n], BF16, tag=f"qkB{m}", name=f"qkB{m}") for m in range(4)]
        for m in range(4):
            for j in range(n // 512):
                acc = ps.tile([P, 512], F32, tag="sp")
                for k in range(kt):
                    nc.tensor.matmul(
                        acc[:], wqk_sb[k][:, m * P:(m + 1) * P],
                        xT_sb[k][:, j * 512:(j + 1) * 512],
                        start=(k == 0), stop=(k == kt - 1))
                # copy psum -> sbuf, adding per-channel qkv bias
                nc.vector.tensor_scalar_add(
                    qkT[m][:, j * 512:(j + 1) * 512], acc[:], bqk_sb[:, m:m + 1])

        # ---- stage 1b: v natural [n, CL] with ones column per head ----
        # per key-block tile [128, HL*65]; head h at cols [h*65, h*65+65)
        v_aug = [vp.tile([P, HL * VW], BF16 if bf16_attn else F32R, tag=f"va{i}", name=f"va{i}") for i in range(nb)]
        for i in range(nb):
            acc = ps.tile([P, CL], F32, tag="sp")
            for k in range(kt):
                nc.tensor.matmul(
                    acc[:], xT_sb[k][:, i * P:(i + 1) * P], wv_sb[k][:],
                    start=(k == 0), stop=False)
            # + b_v broadcast over rows (K=1 ones matmul)
            nc.tensor.matmul(acc[:], ones_r[0:1, 0:P], bv_sb[:],
                             start=False, stop=True)
            for h in range(HL):
                nc.scalar.copy(
                    v_aug[i][:, h * VW:h * VW + D], acc[:, h * D:(h + 1) * D])
                nc.gpsimd.tensor_copy(
                    v_aug[i][:, h * VW + D:h * VW + VW], ones_f[:, 0:1])

        # ---- stage 2: per-head RMSNorm on qT, kT (in place) ----
        for m in range(4):
            for j in range(n // 512):
                js = slice(j * 512, (j + 1) * 512)
                sq = sqp.tile([P, 512], F32R, tag="sq", bufs=4)
                nc.vector.tensor_mul(sq[:], qkT[m][:, js], qkT[m][:, js])
                for h2 in range(HPB):
                    pr = slice(h2 * D, (h2 + 1) * D)
                    # sum over d broadcast to all 128 partitions in one matmul
                    ssq = ps.tile([P, 512], F32, tag="sp")
                    nc.tensor.matmul(ssq[:], ones_m[pr, :], sq[pr, :],
                                     start=True, stop=True)
                    rms = rp.tile([P, 512], F32, tag="rms")
                    nc.scalar.activation(rms[:], ssq[:], AF.Sqrt,
                                         scale=1.0 / D, bias=eps_sb[:, 0:1])
                    rec = rp.tile([P, 512], F32, tag="rec")
                    nc.vector.reciprocal_approx_fast(rec[:], rms[:])
                    # qkT = (qkT * qk_weight_col) * (1/rms)
                    dst = qkB[m] if bf16_attn else qkT[m]
                    nc.vector.scalar_tensor_tensor(
                        dst[pr, js], qkT[m][pr, js], qkw_sb[pr, m:m + 1],
                        rec[pr, :], op0=mybir.AluOpType.mult,
                        op1=mybir.AluOpType.mult)

        if debug_dump:
            for m in range(4):
                nc.sync.dma_start(dqk_d[m], qkT[m][:].bitcast(F32))
            for i in range(nb):
                nc.sync.dma_start(dva_d[i], v_aug[i][:].bitcast(F32))

        # ---- stage 3 + 4: attention per (query block, head), then projection ----
        # xT / stage-1+2 scratch are dead now; release their SBUF for the
        # attention-stage pools
        stg1.close()
        ptp = ctx.enter_context(tc.tile_pool(name="ptp", bufs=20))
        atp = ctx.enter_context(tc.tile_pool(name="atp", bufs=1))
        rp2 = ctx.enter_context(tc.tile_pool(name="rp2", bufs=4))
        osp = ctx.enter_context(tc.tile_pool(name="osp", bufs=4))
        attnT = [atp.tile([P, n], F32R, tag=f"at{t}", name=f"at{t}") for t in range(CL // P)]
        # units are (query-block, head-PAIR): the even head's kT/qT live at
        # partition offset 0, the odd head's at offset 64, so consecutive S
        # matmuls target alternating PE array tiles (row 0 / row 64) and
        # stream concurrently (~2x single-position throughput)
        units = [(j, hp) for j in range(nj) for hp in range(HL // HPB)]

        def emit_s_pair(u, i):
            """S^T matmuls for both heads of the pair at key-block i + exp."""
            j, hp = u
            js = slice(j * nq, (j + 1) * nq)
            qm, km = hp, 2 + hp
            qsrc = qkB if bf16_attn else qkT
            s2 = ps.tile([P, 2 * nq], F32, tag="sp", name="s2")
            for sub in range(HPB):
                pr = slice(sub * D, (sub + 1) * D)
                nc.tensor.matmul(
                    s2[:, sub * nq:(sub + 1) * nq],
                    qsrc[km][pr, i * P:(i + 1) * P], qsrc[qm][pr, js],
                    start=True, stop=True)
            pt = ptp.tile([P, 2 * nq], BF16 if bf16_attn else F32R, tag="pt", name="pt")
            nc.scalar.activation(pt[:], s2[:], AF.Exp, scale=SCALE)
            return pt

        def emit_av(u, oas, pts, i):
            j, hp = u
            for sub in range(HPB):
                h = hp * HPB + sub
                nc.tensor.matmul(
                    oas[sub][:], v_aug[i][:, h * VW:(h + 1) * VW],
                    pts[i][:, sub * nq:(sub + 1) * nq],
                    start=(i == 0), stop=(i == nb - 1))

        def emit_norm(u, oas):
            j, hp = u
            js = slice(j * nq, (j + 1) * nq)
            for sub in range(HPB):
                h = hp * HPB + sub
                oa = oas[sub]
                sums_sb = rp2.tile([1, nq], F32R, tag="sums", name="sums")
                nc.vector.tensor_copy(sums_sb[:], oa[D:VW, :])
                bc = ps.tile([D, nq], F32, tag="ob", bufs=4, name="bc")
                nc.tensor.matmul(bc[:], ones_r[0:1, 0:D], sums_sb[:],
                                 start=True, stop=True)
                rec = rp2.tile([D, nq], F32, tag="recw", name="recw")
                nc.vector.reciprocal_approx_fast(rec[:], bc[:])
                t = (h * D) // P
                prA = slice((h * D) % P, (h * D) % P + D)
                nc.vector.tensor_mul(attnT[t][prA, js], oa[0:D, :], rec[:])

        def emit_proj(j):
            for j2 in range(j * (nq // P), (j + 1) * (nq // P)):
                for half in range(2):
                    hs = slice(half * 512, (half + 1) * 512)
                    acc = ps.tile([P, 512], F32, tag="ob", bufs=4, name="acc")
                    for t in range(CL // P):
                        nc.tensor.matmul(
                            acc[:], attnT[t][:, j2 * P:(j2 + 1) * P],
                            wpr_sb[t][:, hs], start=(t == 0), stop=False)
                    nc.tensor.matmul(acc[:], ones_r[0:1, 0:P], bpr_sb[:, hs],
                                     start=False, stop=True)
                    ost = osp.tile([P, 512], F32, tag="ost", name="ost")
                    nc.vector.tensor_copy(ost[:], acc[:])
                    nc.sync.dma_start(out_d[j2 * P:(j2 + 1) * P, hs], ost[:])

        # software pipeline: S pairs of unit u+1 interleave with AV of unit u
        prev = None   # (unit, pts)
        for idx in range(len(units) + 1):
            cur = units[idx] if idx < len(units) else None
            pts = []
            oas_prev = None
            if prev is not None:
                oas_prev = [ps.tile([VW, nq], F32, tag="ob", bufs=4,
                                    name=f"oa{s_}") for s_ in range(HPB)]
            BLK = 4
            for ib in range(nb // BLK):
                if cur is not None:
                    for i in range(ib * BLK, (ib + 1) * BLK):
                        pts.append(emit_s_pair(cur, i))
                if prev is not None:
                    for i in range(ib * BLK, (ib + 1) * BLK):
                        emit_av(prev[0], oas_prev, prev[1], i)
            if prev is not None:
                emit_norm(prev[0], oas_prev)
                jprev, hpprev = prev[0]
                if hpprev == HL // HPB - 1:
                    emit_proj(jprev)
            prev = (cur, pts) if cur is not None else None

        if debug_dump:
            for t in range(CL // P):
                nc.sync.dma_start(dat_d[t], attnT[t][:].bitcast(F32))

    nc.compile()
    return nc


_NC_CACHE = {}


def _get_nc(n=N, nq=NQ):
    key = (n, nq)
    if key not in _NC_CACHE:
        _NC_CACHE[key] = build(n, nq)
    return _NC_CACHE[key]


def make_in_maps(x, w_qkv, b_qkv, q_w, k_w, w_proj, b_proj):
    """Shard full inputs into per-core in_maps (host side)."""
    n = x.shape[1]
    in_maps = []
    for cid in range(NCORES):
        b, g = cid // GROUPS, cid % GROUPS
        c0 = g * CL
        xT = np.ascontiguousarray(x[b].T)
        w_qk = np.ascontiguousarray(
            np.concatenate([w_qkv[:, c0:c0 + CL],
                            w_qkv[:, C + c0:C + c0 + CL]], axis=1))
        w_v = np.ascontiguousarray(w_qkv[:, 2 * C + c0:2 * C + c0 + CL])
        w_pr = np.ascontiguousarray(w_proj[c0:c0 + CL, :])
        b_qk = np.stack([b_qkv[c0 + m * P:c0 + (m + 1) * P] for m in range(2)]
                        + [b_qkv[C + c0 + m * P:C + c0 + (m + 1) * P]
                           for m in range(2)], axis=1)
        b_v = b_qkv[2 * C + c0:2 * C + c0 + CL].reshape(1, CL)
        # host gather sums GROUPS partials per batch; split the bias so it
        # lands exactly once
        b_pr = (b_proj / GROUPS).reshape(1, C)
        qkw = np.stack([np.tile(q_w, HPB), np.tile(q_w, HPB),
                        np.tile(k_w, HPB), np.tile(k_w, HPB)], axis=1)
        in_maps.append({
            "xT": xT.astype(np.float32),
            "w_qk": w_qk.astype(np.float32),
            "w_v": w_v.astype(np.float32),
            "w_pr": w_pr.astype(np.float32),
            "b_qk": np.ascontiguousarray(b_qk).astype(np.float32),
            "b_v": b_v.astype(np.float32),
            "b_pr": b_pr.astype(np.float32),
            "qkw": np.ascontiguousarray(qkw).astype(np.float32),
        })
    return in_maps


def kernel(x, w_qkv, b_qkv, q_w, k_w, w_proj, b_proj, _trace=False):
    x = np.asarray(x)
    n = x.shape[1]
    nc = _get_nc(n, NQ if n % NQ == 0 else P)
    in_maps = make_in_maps(np.asarray(x, np.float32), np.asarray(w_qkv, np.float32),
                           np.asarray(b_qkv, np.float32), np.asarray(q_w, np.float32),
                           np.asarray(k_w, np.float32), np.asarray(w_proj, np.float32),
                           np.asarray(b_proj, np.float32))
    res = run_bass_kernel_spmd(nc, in_maps, core_ids=list(range(NCORES)),
                               trace=_trace)
    # TP unshard: sum the 4 head-group partials per batch, stack batches
    out = np.stack([
        sum(res.results[b * GROUPS + g]["out"] for g in range(GROUPS))
        for b in range(B)
    ]).astype(np.float32)
    if _trace:
        return out, res
    return out



# revision 19
# speedup vs baseline: 1.5551x; 1.5551x over previous
"""Multi-head attention (B=2, N=2048, C=1024, H=16, qk-RMSNorm) on 8 TRN2 cores.

Sharding: tensor-parallel over heads x data-parallel over batch.
Core cid handles batch b = cid // 4 and head group g = cid % 4 (4 heads,
c_local = 256 channels). Each core computes qkv for its heads, per-head
RMSNorm on q/k, full softmax attention for its (b, heads), and a partial
output projection against its 256 rows of w_proj. The host sums the 4
partials per batch (TP unshard), adds b_proj and the v-bias contribution
(softmax weights sum to 1, so b_v passes through attention additively:
out += b_v @ w_proj), and stacks the 2 batches.

Precision plan (rel-err budget 2e-2, measured ~1e-2):
- weights / x / v / probs / attnT / output partials: bf16 (halves DMA+SBUF,
  enables fast-weight-load on the PE; matmul MAC rate is dtype-independent)
- q/k kept fp32r through RMSNorm and the S matmul (logit noise dominates
  the error budget otherwise)
- softmax exp: split between the Scalar engine (table exp) and the Vector
  engine (Schraudolph bit-trick: y*A+B -> int16 round -> bitcast bf16,
  ~3% max rel err on probs) so neither engine starves the PE. The ones
  column appended to V gives denominators from the same AV matmuls; the
  denominator uses the same approximated exps, so common-mode error
  cancels.

Engine balance per attention unit (2 heads x 512 queries, 16 key blocks):
PE ~12us (S pairs on alternating 64-row groups run concurrently; AV M=65;
1 broadcast matmul; projection share), ACT ~12us (12 exp tiles), DVE ~10us
(4 exp tiles + normalization chain + PSUM evacuations). Keeping every
engine under the PE's per-unit time keeps the PE busy and the HAM clock
gate at 8/8 (the fp32 baseline lost ~2x to ACT-starved PE idle windows).
"""

import sys

if "/opt/trn_rl_repo" not in sys.path:
    sys.path.insert(0, "/opt/trn_rl_repo")

from contextlib import ExitStack

import numpy as np
import ml_dtypes

import concourse.mybir as mybir
import concourse.tile as tile
from concourse import bacc
from concourse.bass_utils import run_bass_kernel_spmd

F32 = mybir.dt.float32
F32R = mybir.dt.float32r
BF16 = mybir.dt.bfloat16
I16 = mybir.dt.int16
AF = mybir.ActivationFunctionType
ALU = mybir.AluOpType

# Problem constants (hardcoded per contract)
B, N, C, H = 2, 2048, 1024, 16
D = C // H          # 64
EPS = 1e-6
NCORES = 8
GROUPS = 4          # head groups (cores per batch)
HL = H // GROUPS    # heads per core = 4
CL = HL * D         # local channels = 256
SCALE = D ** -0.5   # 0.125

# Tiling
P = 128             # partition dim
KT = C // P         # 8 contraction tiles over C
NQ = 512            # query-block (free dim of S^T / AV matmuls)
NB = N // P         # 16 key blocks of 128
HPB = P // D        # heads per 128-channel block = 2
VW = D + 1          # 65: v columns + ones column

# Schraudolph exp via int16: bf16(exp(y)) ~= int16(y*A16 + B16) bitcast bf16
A16 = (2.0 ** 23 / np.log(2.0)) / 65536.0   # 184.6650...
B16 = 16256.0 - 334500.0 / 65536.0          # tuned for minimax rel err ~3.3%
# which key blocks (mod 4) go to the DVE exp path; rest go to ACT
DVE_EXP_MOD = (3,)


def build(n=N, nq=NQ):
    """Build the SPMD Bass module. n = sequence length (for scaled tests)."""
    nb = n // P
    nj = n // nq
    kt = KT

    nc = bacc.Bacc("TRN2", target_bir_lowering=False, debug=False,
                   num_devices=NCORES)

    xT_d = nc.dram_tensor("xT", [C, n], BF16, kind="ExternalInput").ap()
    wqk_d = nc.dram_tensor("w_qk", [C, 2 * CL], BF16, kind="ExternalInput").ap()
    wv_d = nc.dram_tensor("w_v", [C, CL], BF16, kind="ExternalInput").ap()
    wpr_d = nc.dram_tensor("w_pr", [CL, C], BF16, kind="ExternalInput").ap()
    bqk_d = nc.dram_tensor("b_qk", [P, 4], F32, kind="ExternalInput").ap()
    qkw_d = nc.dram_tensor("qkw", [P, 4], F32, kind="ExternalInput").ap()
    onesd_d = nc.dram_tensor("onesd", [P, P], BF16, kind="ExternalInput").ap()
    ones2_d = nc.dram_tensor("ones2", [2, P], F32, kind="ExternalInput").ap()
    out_d = nc.dram_tensor("out", [n, C], BF16, kind="ExternalOutput").ap()

    with tile.TileContext(nc) as tc, ExitStack() as ctx:
        con = ctx.enter_context(tc.tile_pool(name="con", bufs=1))
        wp = ctx.enter_context(tc.tile_pool(name="wp", bufs=1))
        qk = ctx.enter_context(tc.tile_pool(name="qk", bufs=1))
        vp = ctx.enter_context(tc.tile_pool(name="vp", bufs=1))
        xp = ctx.enter_context(tc.tile_pool(name="xp", bufs=1))
        sqp = ctx.enter_context(tc.tile_pool(name="sqp", bufs=4))
        rp = ctx.enter_context(tc.tile_pool(name="rp", bufs=4))
        ps = ctx.enter_context(tc.tile_pool(name="ps", bufs=2, space="PSUM"))
        ob = ctx.enter_context(tc.tile_pool(name="ob", bufs=4, space="PSUM"))
        ptp = ctx.enter_context(tc.tile_pool(name="ptp", bufs=20))
        atp = ctx.enter_context(tc.tile_pool(name="atp", bufs=1))
        rp2 = ctx.enter_context(tc.tile_pool(name="rp2", bufs=4))
        osp = ctx.enter_context(tc.tile_pool(name="osp", bufs=4))

        # ---- constants ----
        # block-diag ones (64x64 blocks) as lhsT: one matmul broadcasts each
        # head's partition-sum of squares back to that head's 64 partitions
        onesd = con.tile([P, P], BF16, tag="onesd")
        nc.sync.dma_start(onesd[:], onesd_d[:])
        # 2-row block mask: row0 -> cols 0:64, row1 -> cols 64:128 (denominator
        # broadcast for both heads of a pair in one K=2 matmul)
        ones2 = con.tile([2, P], F32R, tag="ones2")
        nc.sync.dma_start(ones2[:], ones2_d[:].bitcast(F32R))

        eps_sb = con.tile([P, 1], F32, tag="eps")
        nc.vector.memset(eps_sb[:], EPS)

        bqk_sb = con.tile([P, 4], F32, tag="bqk")
        nc.sync.dma_start(bqk_sb[:], bqk_d[:])
        qkw_sb = con.tile([P, 4], F32, tag="qkw")
        nc.sync.dma_start(qkw_sb[:], qkw_d[:])

        # ---- weight / input loads (x and w_qk first: stage 1a gates on them)
        wqk_sb = [wp.tile([P, 2 * CL], BF16, tag=f"wqk{k}", name=f"wqk{k}") for k in range(kt)]
        wv_sb = [wp.tile([P, CL], BF16, tag=f"wv{k}", name=f"wv{k}") for k in range(kt)]
        wpr_sb = [wp.tile([P, C], BF16, tag=f"wpr{k}", name=f"wpr{k}") for k in range(CL // P)]
        xT_sb = [xp.tile([P, n], BF16, tag=f"xt{k}", name=f"xt{k}") for k in range(kt)]
        for k in range(kt):
            nc.sync.dma_start(xT_sb[k][:], xT_d[k * P:(k + 1) * P, :])
            nc.sync.dma_start(wqk_sb[k][:], wqk_d[k * P:(k + 1) * P, :])
        for k in range(kt):
            nc.sync.dma_start(wv_sb[k][:], wv_d[k * P:(k + 1) * P, :])
        for k in range(CL // P):
            nc.sync.dma_start(wpr_sb[k][:], wpr_d[k * P:(k + 1) * P, :])

        # ---- stage 1a+2: qkT = (x @ w_qk)^T, biased + per-head RMSNorm ----
        # 4 channel blocks of 128: q(heads01), q(heads23), k(heads01), k(heads23)
        # k blocks (m=2,3) first: the attention units need all of k but only
        # one q block each.
        qkB = [qk.tile([P, n], F32R, tag=f"qkB{m}", name=f"qkB{m}") for m in range(4)]
        for m in (2, 3, 0, 1):
            for j in range(n // 512):
                js = slice(j * 512, (j + 1) * 512)
                acc = ps.tile([P, 512], F32, tag="sp", name="acc")
                for k in range(kt):
                    nc.tensor.matmul(
                        acc[:], wqk_sb[k][:, m * P:(m + 1) * P],
                        xT_sb[k][:, js],
                        start=(k == 0), stop=(k == kt - 1))
                # evacuate with bias add (DVE: f32r out), square with bias (ACT)
                nc.vector.tensor_scalar_add(qkB[m][:, js], acc[:],
                                            bqk_sb[:, m:m + 1])
                sq = sqp.tile([P, 512], BF16, tag="sq", name="sq")
                nc.scalar.activation(sq[:], acc[:], AF.Square,
                                     bias=bqk_sb[:, m:m + 1])
                # per-head sum of squares, broadcast to the head's partitions
                ssq = ob.tile([P, 512], F32, tag="ob", name="ssq", space="PSUM")
                nc.tensor.matmul(ssq[:], onesd[:], sq[:], start=True, stop=True)
                rms = rp.tile([P, 512], F32, tag="rms", name="rms")
                nc.scalar.activation(rms[:], ssq[:], AF.Sqrt,
                                     scale=1.0 / D, bias=eps_sb[:, 0:1])
                rec = rp.tile([P, 512], F32, tag="rec", name="rec")
                nc.vector.reciprocal_approx_fast(rec[:], rms[:])
                # qkB = (qkB * qk_weight_col) * (1/rms), in place
                nc.vector.scalar_tensor_tensor(
                    qkB[m][:, js], qkB[m][:, js], qkw_sb[:, m:m + 1],
                    rec[:], op0=ALU.mult, op1=ALU.mult)

        # ---- stage 1b: v natural [n, HL, VW] with ones column per head ----
        v_aug = [vp.tile([P, HL, VW], BF16, tag=f"va{i}", name=f"va{i}") for i in range(nb)]
        for i in range(nb):
            vacc = ob.tile([P, CL], F32, tag="ob", name="vacc", space="PSUM")
            for k in range(kt):
                nc.tensor.matmul(
                    vacc[:], xT_sb[k][:, i * P:(i + 1) * P], wv_sb[k][:],
                    start=(k == 0), stop=(k == kt - 1))
            nc.vector.memset(v_aug[i][:, :, D:VW], 1.0)
            nc.scalar.activation(
                v_aug[i][:, :, 0:D],
                vacc[:].rearrange("p (h x) -> p h x", h=HL), AF.Copy)

        # ---- stage 3 + 4: attention per (query block, head pair) + proj ----
        attnT = [atp.tile([P, n], BF16, tag=f"at{t}", name=f"at{t}") for t in range(CL // P)]
        # units are (query-block, head-PAIR): the even head's kT/qT live at
        # partition offset 0, the odd head's at offset 64, so consecutive S
        # matmuls target alternating PE array row groups and run concurrently
        units = [(j, hp) for j in range(nj) for hp in range(HL // HPB)]

        def emit_s_pair(u, i):
            """S^T matmuls for both heads of the pair at key-block i + exp."""
            j, hp = u
            js = slice(j * nq, (j + 1) * nq)
            qm, km = hp, 2 + hp
            s2 = ps.tile([P, 2 * nq], F32, tag="sp", name="s2")
            for sub in range(HPB):
                pr = slice(sub * D, (sub + 1) * D)
                nc.tensor.matmul(
                    s2[:, sub * nq:(sub + 1) * nq],
                    qkB[km][pr, i * P:(i + 1) * P], qkB[qm][pr, js],
                    start=True, stop=True)
            pt = ptp.tile([P, 2 * nq], BF16, tag="pt", name="pt")
            if (i % 4) in DVE_EXP_MOD:
                # Schraudolph exp on the DVE: round(y*A+B) as int16 == bf16 bits
                nc.vector.tensor_scalar(
                    pt[:].bitcast(I16), s2[:],
                    float(SCALE * A16), float(B16), ALU.mult, ALU.add)
            else:
                nc.scalar.activation(pt[:], s2[:], AF.Exp, scale=SCALE)
            return pt

        def emit_av(u, oas, pts, i):
            j, hp = u
            for sub in range(HPB):
                h = hp * HPB + sub
                nc.tensor.matmul(
                    oas[sub][:], v_aug[i][:, h, :],
                    pts[i][:, sub * nq:(sub + 1) * nq],
                    start=(i == 0), stop=(i == nb - 1))

        def emit_norm(u, oas):
            j, hp = u
            js = slice(j * nq, (j + 1) * nq)
            # per-head denominator row -> K=1 broadcast matmul -> reciprocal
            for sub in range(HPB):
                pr = slice(sub * D, (sub + 1) * D)
                sums = rp2.tile([1, nq], F32R, tag="sums", name="sums")
                nc.vector.tensor_copy(sums[:], oas[sub][D:VW, :])
                bc = ob.tile([D, nq], F32, tag="ob", name="bc", space="PSUM")
                nc.tensor.matmul(bc[:], ones2[0:1, 0:D], sums[:],
                                 start=True, stop=True)
                rec = rp2.tile([D, nq], F32, tag="recw", name="recw")
                nc.vector.reciprocal_approx_fast(rec[:], bc[:])
                nc.vector.tensor_mul(attnT[hp][pr, js], oas[sub][0:D, :],
                                     rec[:])

        def emit_proj(j):
            for jj, j2 in enumerate(range(j * (nq // P), (j + 1) * (nq // P))):
                for half in range(2):
                    hs = slice(half * 512, (half + 1) * 512)
                    acc = ob.tile([P, 512], F32, tag="ob", name="acc",
                                  space="PSUM")
                    for t in range(CL // P):
                        nc.tensor.matmul(
                            acc[:], attnT[t][:, j2 * P:(j2 + 1) * P],
                            wpr_sb[t][:, hs], start=(t == 0),
                            stop=(t == CL // P - 1))
                    ost = osp.tile([P, 512], BF16, tag="ost", name="ost")
                    nc.vector.tensor_copy(ost[:], acc[:])
                    nc.sync.dma_start(out_d[j2 * P:(j2 + 1) * P, hs], ost[:])

        # software pipeline: S pairs of unit u+1 interleave with AV of unit u
        prev = None   # (unit, pts)
        for idx in range(len(units) + 1):
            cur = units[idx] if idx < len(units) else None
            pts = []
            oas_prev = None
            if prev is not None:
                oas_prev = [ob.tile([VW, nq], F32, tag="ob", bufs=4,
                                    space="PSUM", name=f"oa{s_}")
                            for s_ in range(HPB)]
            BLK = 4
            for ib in range(nb // BLK):
                if cur is not None:
                    for i in range(ib * BLK, (ib + 1) * BLK):
                        pts.append(emit_s_pair(cur, i))
                if prev is not None:
                    for i in range(ib * BLK, (ib + 1) * BLK):
                        emit_av(prev[0], oas_prev, prev[1], i)
            if prev is not None:
                emit_norm(prev[0], oas_prev)
                jprev, hpprev = prev[0]
                if hpprev == HL // HPB - 1:
                    emit_proj(jprev)
            prev = (cur, pts) if cur is not None else None

    nc.compile()
    return nc


_NC_CACHE = {}


def _get_nc(n=N, nq=NQ):
    key = (n, nq)
    if key not in _NC_CACHE:
        _NC_CACHE[key] = build(n, nq)
    return _NC_CACHE[key]


def make_in_maps(x, w_qkv, b_qkv, q_w, k_w, w_proj, b_proj):
    """Shard full inputs into per-core in_maps (host side)."""
    bf = ml_dtypes.bfloat16
    in_maps = []
    for cid in range(NCORES):
        b, g = cid // GROUPS, cid % GROUPS
        c0 = g * CL
        xT = np.ascontiguousarray(x[b].T)
        w_qk = np.ascontiguousarray(
            np.concatenate([w_qkv[:, c0:c0 + CL],
                            w_qkv[:, C + c0:C + c0 + CL]], axis=1))
        w_v = np.ascontiguousarray(w_qkv[:, 2 * C + c0:2 * C + c0 + CL])
        w_pr = np.ascontiguousarray(w_proj[c0:c0 + CL, :])
        b_qk = np.stack([b_qkv[c0 + m * P:c0 + (m + 1) * P] for m in range(2)]
                        + [b_qkv[C + c0 + m * P:C + c0 + (m + 1) * P]
                           for m in range(2)], axis=1)
        qkw = np.stack([np.tile(q_w, HPB), np.tile(q_w, HPB),
                        np.tile(k_w, HPB), np.tile(k_w, HPB)], axis=1)
        onesd = np.zeros((P, P), np.float32)
        onesd[:D, :D] = 1.0
        onesd[D:, D:] = 1.0
        ones2 = np.zeros((2, P), np.float32)
        ones2[0, :D] = 1.0
        ones2[1, D:] = 1.0
        in_maps.append({
            "xT": xT.astype(bf),
            "w_qk": w_qk.astype(bf),
            "w_v": w_v.astype(bf),
            "w_pr": w_pr.astype(bf),
            "b_qk": np.ascontiguousarray(b_qk).astype(np.float32),
            "qkw": np.ascontiguousarray(qkw).astype(np.float32),
            "onesd": onesd.astype(bf),
            "ones2": ones2,
        })
    return in_maps


def kernel(x, w_qkv, b_qkv, q_w, k_w, w_proj, b_proj, _trace=False):
    x = np.asarray(x, np.float32)
    w_qkv = np.asarray(w_qkv, np.float32)
    b_qkv = np.asarray(b_qkv, np.float32)
    w_proj = np.asarray(w_proj, np.float32)
    b_proj = np.asarray(b_proj, np.float32)
    n = x.shape[1]
    nc = _get_nc(n, NQ)
    in_maps = make_in_maps(x, w_qkv, np.asarray(b_qkv, np.float32),
                           np.asarray(q_w, np.float32),
                           np.asarray(k_w, np.float32), w_proj, b_proj)
    res = run_bass_kernel_spmd(nc, in_maps, core_ids=list(range(NCORES)),
                               trace=_trace)
    # v-bias passes through softmax additively; add its projection (and
    # b_proj) once per batch on the host
    bias_row = (b_qkv[2 * C:] @ w_proj + b_proj).astype(np.float32)
    # TP unshard: sum the 4 head-group partials per batch, stack batches
    out = np.stack([
        sum(res.results[b * GROUPS + g]["out"].astype(np.float32)
            for g in range(GROUPS)) + bias_row
        for b in range(B)
    ]).astype(np.float32)
    if _trace:
        return out, res
    return out
